# revision 34
# baseline (speedup 1.0000x reference)
"""Trainium2 Bass kernel for nn_FC_LSTM (FC-LSTM encoder-decoder).

Strategy (v2):
  - Data-parallel over batch: 256 samples -> 8 cores x 32 samples.
  - Feature-major layout on chip: activations [feature(part), sample(free)].
  - ENCODER entirely in fp8e4m3 with DoubleRow matmuls (0.5 cyc/row):
    FC1 (4096->1024), FC2 (1024->256), and all three encoder LSTM cells.
    Scale algebra chosen so every FC epilogue is a 2-op DVE tensor_scalar
    (add bias, max 0) and cell gates use the Act `scale` operand:
      w1q=16*w1, x raw     -> psum1 = 16*(w1@x);  z1q = 16*z1
      w2q=4*w2             -> psum2 = 64*(w2@z1); zq  = 64*z
      en1 wihq=16*wih (x zq), en2/en3 wihq=1024*wih (x hq raw),
      whhq=1024*whh, cell psum scale 1024 -> Act sigmoid/tanh(scale=1/1024)
    h state stored as raw fp8e4 (validated: rel err 2.3e-3 incl fp16 out).
  - en1's input-side gate matmul runs IN-CELL (accumulated into the gate
    PSUM) instead of a precomputed dense gi + DVE add: drops a DVE-PSUM
    add from the recurrence critical path.
  - DECODER stays bf16 (fp8 there costs ~1e-2 rel err). FC-D restructured
    to PER-STEP chunks: wd1 into a packed [128, 8m x 32] one-bank psum,
    wd2 into two [128, 16m x 32] banks with per-m Act tanh(+bias) -> fp16
    output tile -> one DMA per block. FC-D(t) work is emitted right after
    cell de3(t), so decoder filler exists from step 0 (the old 5-step-chunk
    scheme left decoder steps 0-4 with zero PE filler).
  - Cell biases injected via the K=128 indicator matmul (E @ bB) as before.
  - Output DMA'd as fp16 (halves output traffic; y in [-0.09, 0.09]).
  - LSTM recurrence keeps gates packed in one PSUM bank [128, 8*32], gate
    order host-permuted to [i, f, o, g]: sigmoid cols 0..191, tanh 192..255.
  - FC phases emitted as generators whose matmul blocks interleave into the
    recurrence as PE gap fillers; cells software-pipelined (en2(t),
    en1(t+1), en3(t)).
"""

import os
import time
from contextlib import ExitStack

import ml_dtypes
import numpy as np

import concourse.bass as bass
import concourse.mybir as mybir
import concourse.tile as tile

BF16NP = ml_dtypes.bfloat16
FP8NP = ml_dtypes.float8_e4m3
AF = mybir.ActivationFunctionType
DT = mybir.dt
DR = mybir.MatmulPerfMode.DoubleRow
ALU = mybir.AluOpType

S = 20          # encoder sequence length
B = 256         # global batch
NCORES = 8
BL = B // NCORES  # 32 samples per core
H = 256         # LSTM hidden
G = 4 * H       # 1024 gate features
D = 4096        # input feature dim (64*64)
HID = 1024      # FC hidden
SB = S * BL     # 640 encoder samples per core

# fp8 scale algebra
SW1 = 16.0      # w1 weight scale
SZ1 = 16.0      # z1 activation scale (= SW1 * 1)
SW2 = 4.0       # w2 weight scale
SZ = 64.0       # z activation scale (= SW2 * SZ1)
SCELL = 1024.0  # encoder cell psum scale

VERBOSE = True
EPI_ENGINE = os.environ.get("EPI", "dve")      # FC epilogue engine
CADD_ENGINE = os.environ.get("CADD", "dve")    # cell c-add engine
CMUL_ENGINE = os.environ.get("CMUL", "pool")   # cell c-mul engine
FILL_PLACE = os.environ.get("FILLP", "pre")  # 'pre' or 'inter'


def _log(*a):
    if VERBOSE:
        print("[kernel]", *a, flush=True)


# ---------------------------------------------------------------------------
# Workaround: walrus CoreV3 setupSyncWait allows only 1 sync wait on the
# TileContext exit Drain. Split its waits across multiple drain instructions.
# ---------------------------------------------------------------------------
def _patched_drain_and_barrier(self, tick_clock, wait_clock):
    nc = self.nc
    drain_inst = nc.sync.drain()
    wait_clock.add_sem_waits(
        drain_inst.ins, tile.ScopedClock({None: tick_clock.global_clock})
    )
    inst = drain_inst.ins
    si = inst.sync_info
    waits = list(si.on_wait) if si is not None and si.on_wait else []
    MAXW = 1
    if len(waits) > MAXW:
        si.on_wait = waits[:MAXW]
        for i in range(MAXW, len(waits), MAXW):
            d2 = nc.sync.drain()
            i2 = d2.ins
            si2 = i2.sync_info
            if si2 is None:
                i2.sync_info = type(si)(on_wait=waits[i : i + MAXW], on_update=[])
            else:
                si2.on_wait = list(si2.on_wait or []) + waits[i : i + MAXW]

    nc.all_engine_barrier()
    assert self.sems is not None
    popped = nc._tile_sem_poison_stack.pop()
    assert popped is self._sem_poison
    nc.clear_and_free_semaphores(list(self.sems.allocated().values()))
    nc.all_engine_barrier()


tile.TileContext._drain_and_barrier = _patched_drain_and_barrier


def _split_sync_waits(nc, limit=1):
    """walrus setupSyncWait rejects >2 sem waits per instruction: move excess
    waits onto same-engine NoOps spliced just before the instruction."""
    ctr = [0]
    SyncInfo = None
    for f in nc.m.functions:
        for bb in f.blocks:
            out = []
            for inst in bb.instructions:
                si = inst.sync_info
                waits = list(si.on_wait) if si is not None and si.on_wait else []
                if len(waits) > limit:
                    if SyncInfo is None:
                        SyncInfo = type(si)
                    extras = waits[: len(waits) - limit]
                    si.on_wait = waits[len(waits) - limit:]
                    for i in range(0, len(extras), limit):
                        ctr[0] += 1
                        nop = mybir.InstNoOp(name=f"ws_{ctr[0]}", ins=[], outs=[])
                        nop.engine = inst.engine
                        nop.sync_info = SyncInfo(
                            on_wait=extras[i : i + limit], on_update=[]
                        )
                        out.append(nop)
                out.append(inst)
            bb.instructions[:] = out
    return ctr[0]


# ---------------------------------------------------------------------------
# Program builder
# ---------------------------------------------------------------------------
EN_CELLS = ["en1", "en2", "en3"]
DE_CELLS = ["de1", "de2", "de3"]
KP1 = D // 256       # 16 k-pairs for FC1
KP2 = HID // 256     # 4 k-pairs for FC2
NCHA = 4             # FC-A chunks (5 encoder steps each)
CHA = SB // NCHA     # 160 samples
SPC = S // NCHA      # steps per chunk


def build_program(F: int, nrep: int = 1) -> bass.Bass:
    FB = F * BL  # decoder samples per core
    nc = bass.Bass()

    # --- DRAM tensors (merged: HWDGE charges a flat 625ns per DMA) ---
    # x: chunk-major k-pair layout -> one DMA per chunk
    xq = nc.dram_tensor("xq", [128, NCHA, KP1, 2, CHA], DT.float8e4,
                        kind="ExternalInput")
    # w1: 4 groups of 4 k-pairs for k-outer pacing of chunk 0
    w1q = nc.dram_tensor("w1q", [128, KP1, 2, HID], DT.float8e4,
                         kind="ExternalInput")
    w2q = nc.dram_tensor("w2q", [128, KP2, 2, H], DT.float8e4,
                         kind="ExternalInput")
    # encoder cells: wih+whh packed per cell
    enw = {nm: nc.dram_tensor(f"{nm}_w", [128, 2, 2, G], DT.float8e4,
                              kind="ExternalInput")
           for nm in EN_CELLS}
    # decoder cell weights, fp8, packed: de1_whh, de2_wih, de2_whh,
    # de3_wih, de3_whh
    dew = nc.dram_tensor("dew", [128, 5, 2, G], DT.float8e4,
                         kind="ExternalInput")
    wd1m = nc.dram_tensor("wd1m", [128, 2, HID], DT.bfloat16,
                          kind="ExternalInput")
    # bf16 constant blob: Em(256) E16(512) bB x6 (768) bd2B(256)
    cbf = nc.dram_tensor("cbf", [128, 1792], DT.bfloat16, kind="ExternalInput")
    # f32 bias blob: b1(8) b2(2) bd1(8)
    cf32 = nc.dram_tensor("cf32", [128, 18], DT.float32, kind="ExternalInput")
    wd2T = nc.dram_tensor("wd2T", [HID, D], DT.bfloat16, kind="ExternalInput")
    yT = nc.dram_tensor("yT", [D, FB], DT.float16, kind="ExternalOutput")

    epi_eng = nc.vector if EPI_ENGINE == "dve" else nc.gpsimd
    cadd_eng = nc.vector if CADD_ENGINE == "dve" else nc.gpsimd
    cmul_eng = nc.vector if CMUL_ENGINE == "dve" else nc.gpsimd

    with tile.TileContext(nc) as tc:
     for rep in range(nrep):
      with ExitStack() as ctx:
        const = ctx.enter_context(tc.tile_pool(name="const", bufs=1))
        state = ctx.enter_context(tc.tile_pool(name="state", bufs=3))
        gates = ctx.enter_context(tc.tile_pool(name="gates", bufs=4))
        outp = ctx.enter_context(tc.tile_pool(name="outp", bufs=4))
        psum = ctx.enter_context(tc.tile_pool(name="psum", bufs=5,
                                              space="PSUM"))
        psumD = ctx.enter_context(tc.tile_pool(name="psumD", bufs=3,
                                               space="PSUM"))

        uid = [0]

        def PS():
            uid[0] += 1
            return psum.tile([128, 512], DT.float32, tag="ps",
                             name=f"ps{uid[0]}")

        def PSD():
            uid[0] += 1
            return psumD.tile([128, 512], DT.float32, tag="psD",
                              name=f"psD{uid[0]}")

        def dma_in(pool, dram, kshape, tag):
            """Load [K, M] dram weight into [128, K//128, M] sbuf tile."""
            k, m = kshape
            t = pool.tile([128, k // 128, m], dram.dtype, tag=tag)
            nc.sync.dma_start(t[:], dram.rearrange("(o p) m -> p o m", p=128))
            return t

        def dma_in2(pool, dram, tag):
            t = pool.tile(list(dram.shape), dram.dtype, tag=tag)
            nc.sync.dma_start(t[:], dram[:])
            return t

        # ========== DMA emission, ordered by first use ====================
        pA_ctx = ExitStack()
        pA = pA_ctx.enter_context(tc.tile_pool(name="phaseA", bufs=1))

        # x chunk 0 first, then w1 in 4 groups (k-outer pacing of chunk 0)
        x_c = [None] * NCHA

        def dma_x_chunk(c):
            t = pA.tile([128, KP1, 2, CHA], DT.float8e4, tag=f"x_{c}",
                        name=f"x_{c}")
            nc.sync.dma_start(t[:], xq[:, c])
            x_c[c] = t

        x0_t = pA.tile([128, KP1, 2, CHA], DT.float8e4, tag="x_0",
                       name="x_0")
        x_c[0] = x0_t
        w1_g = []

        def dma_w1_g(gi2):
            wk = pA.tile([128, 2, 2, HID], DT.float8e4, tag=f"w1_{gi2}",
                         name=f"w1_{gi2}")
            nc.sync.dma_start(wk[:], w1q[:, 2 * gi2:2 * gi2 + 2])
            w1_g.append(wk)

        nc.sync.dma_start(x0_t[:, 0:8], xq[:, 0, 0:8])
        for gi2 in range(4):
            dma_w1_g(gi2)
        nc.sync.dma_start(x0_t[:, 8:KP1], xq[:, 0, 8:KP1])
        for gi2 in range(4, 8):
            dma_w1_g(gi2)

        def w1_ap(kp):
            return w1_g[kp // 2][:, kp % 2]

        # constants needed early in chunk 0 / first cells
        cf_sb = dma_in2(const, cf32, "cf32")
        b1_sb = cf_sb[:, 0:8]
        b2_sb = cf_sb[:, 8:10]
        bd1_sb = cf_sb[:, 10:18]
        w2_sb = const.tile([128, KP2, 2, H], DT.float8e4, tag="w2")
        nc.sync.dma_start(w2_sb[:], w2q[:])
        cb_sb = dma_in2(const, cbf, "cbf")
        E_sb = cb_sb[:, 0:256]
        E16_sb = cb_sb[:, 256:768]
        bB_sb = {}
        for i, nm in enumerate(EN_CELLS + DE_CELLS):
            bB_sb[nm] = cb_sb[:, 768 + i * 128:768 + (i + 1) * 128]
        bd2B_sb = cb_sb[:, 1536:1792]
        cell_sb = {}
        for nm in EN_CELLS:
            w = const.tile([128, 2, 2, G], DT.float8e4, tag=f"{nm}_w")
            nc.sync.dma_start(w[:], enw[nm][:])
            cell_sb[nm] = {"wih": w[:, 0], "whh": w[:, 1], "bB": bB_sb[nm]}
        dma_x_chunk(1)
        dma_x_chunk(2)
        dma_x_chunk(3)

        # decoder cell weights (fp8) + wd1 (bf16)
        dew_sb = const.tile([128, 5, 2, G], DT.float8e4, tag="dew")
        nc.sync.dma_start(dew_sb[:], dew[:])
        cell_sb["de1"] = {"whh": dew_sb[:, 0], "bB": bB_sb["de1"]}
        cell_sb["de2"] = {"wih": dew_sb[:, 1], "whh": dew_sb[:, 2],
                          "bB": bB_sb["de2"]}
        cell_sb["de3"] = {"wih": dew_sb[:, 3], "whh": dew_sb[:, 4],
                          "bB": bB_sb["de3"]}
        wd1_sb = const.tile([128, 2, HID], DT.bfloat16, tag="wd1")
        nc.sync.dma_start(wd1_sb[:], wd1m[:])

        zh8 = const.tile([128, 2 * BL], DT.float8e4, tag="zh8")
        zc = const.tile([128, 2 * BL], DT.float32, tag="zc")
        nc.vector.memset(zh8[:], 0.0)
        nc.vector.memset(zc[:], 0.0)

        zq_c = [const.tile([128, 2, CHA], DT.float8e4, tag=f"zq_{c}",
                           name=f"zq_{c}")
                for c in range(NCHA)]

        def z_ap(t):
            c, o = divmod(t, SPC)
            return zq_c[c][:, :, o * BL:(o + 1) * BL]

        # ========== Phase A generator =====================================
        def phaseA_gen():
            for c in range(NCHA):
                z1c = pA.tile([128, HID // 128, CHA], DT.float8e4,
                              tag=f"z1_{c}", name=f"z1_{c}")
                if c == 0:
                    # chunk 0 runs while w1/x DMAs land: kp-outer so each
                    # arriving k-pair group is consumed immediately
                    pss = [PS()[:, :CHA] for _ in range(HID // 128)]
                    for kp in range(KP1 - 1):
                        for m in range(HID // 128):
                            nc.tensor.matmul(
                                pss[m], w1_ap(kp)[:, :, m * 128:(m + 1) * 128],
                                x_c[0][:, kp],
                                start=(kp == 0), stop=False,
                                perf_mode=DR,
                            )
                        if kp % 4 == 3:
                            yield c
                    for m in range(HID // 128):
                        nc.tensor.matmul(
                            pss[m], w1_ap(KP1 - 1)[:, :, m * 128:(m + 1) * 128],
                            x_c[0][:, KP1 - 1],
                            start=False, stop=True, perf_mode=DR,
                        )
                        epi_eng.tensor_scalar(
                            z1c[:, m, :], pss[m], b1_sb[:, m:m + 1], 0.0,
                            ALU.add, ALU.max)
                else:
                    for m in range(HID // 128):
                        ps = PS()[:, :CHA]
                        for kp in range(KP1):
                            nc.tensor.matmul(
                                ps, w1_ap(kp)[:, :, m * 128:(m + 1) * 128],
                                x_c[c][:, kp],
                                start=(kp == 0), stop=(kp == KP1 - 1),
                                perf_mode=DR,
                            )
                            if kp % 8 == 7:
                                yield c
                        epi_eng.tensor_scalar(
                            z1c[:, m, :], ps, b1_sb[:, m:m + 1], 0.0,
                            ALU.add, ALU.max)
                # FC2: one half-bank psum per m-tile
                for m in range(H // 128):
                    pm = PS()[:, :CHA]
                    for kp in range(KP2):
                        nc.tensor.matmul(
                            pm, w2_sb[:, kp, :, m * 128:(m + 1) * 128],
                            z1c[:, 2 * kp:2 * kp + 2, :],
                            start=(kp == 0), stop=(kp == KP2 - 1),
                            perf_mode=DR, skip_group_check=True,
                        )
                    epi_eng.tensor_scalar(
                        zq_c[c][:, m, :], pm, b2_sb[:, m:m + 1], 0.0,
                        ALU.add, ALU.max)
                yield c + 1
            while True:
                yield NCHA + 1

        genA = phaseA_gen()
        a_done = [0]

        def fillA(n=1):
            for _ in range(n):
                a_done[0] = max(a_done[0], next(genA))

        def needA(chunks):
            while a_done[0] < chunks + 1:
                fillA()

        # ========== LSTM cells ============================================
        def cell_elem(ps, c_prev, htag, ctag, h_dtype, scale, also_bf16):
            """Shared cell epilogue, split in two phases so the in-order Act
            queue holds [sig, sig, sig, tanh, tanh, tanh] per iteration
            instead of serializing cells on sig->tanh chains.
            The g-gate weight rows are host-doubled so ONE sigmoid covers all
            256 cols: cols 192:256 hold sig(2*pre_g), and tanh(pre_g) =
            2*sig(2*pre_g) - 1 is recovered with a fused DVE op."""
            g = gates.tile([128, 256], DT.float32, tag="g", name=f"g{uid[0]}")
            nc.scalar.activation(g[:], ps, AF.Sigmoid, scale=scale)
            # packed: i: 0..63, f: 64..127, o: 128..191, sg: 192..255
            # t1 = i*tanh(pre_g) = 2*i*sig(2 pre_g) - i  (g rows host-doubled)
            t2 = gates.tile([128, 64], DT.float32, tag="t2", name=f"t2{uid[0]}")
            nc.vector.scalar_tensor_tensor(t2[:], g[:, 192:256], 2.0,
                                           g[:, 0:64], ALU.mult, ALU.mult)
            t1 = gates.tile([128, 64], DT.float32, tag="t1", name=f"t1{uid[0]}")
            nc.vector.tensor_sub(t1[:], t2[:], g[:, 0:64])
            # c path on the (otherwise idle) Pool engine
            c_new = state.tile([128, 64], DT.float32, tag=ctag,
                               name=f"{ctag}{uid[0]}")
            cmul_eng.tensor_mul(c_new[:], g[:, 64:128], c_prev[:])
            cadd_eng.tensor_add(c_new[:], c_new[:], t1[:])

            def finish():
                th = gates.tile([128, 64], DT.float32, tag="th",
                                name=f"th{uid[0]}")
                nc.scalar.activation(th[:], c_new[:], AF.Tanh)
                h_new = state.tile([128, 64], h_dtype, tag=htag,
                                   name=f"{htag}{uid[0]}")
                nc.vector.tensor_mul(h_new[:], g[:, 128:192], th[:])
                h_bf = None
                if also_bf16:
                    h_bf = state.tile([128, 64], DT.bfloat16, tag=htag + "b",
                                      name=f"{htag}b{uid[0]}")
                    nc.vector.tensor_mul(h_bf[:], g[:, 128:192], th[:])
                return h_new, c_new, h_bf
            return finish

        def cell_fp8(nm, xq_ap, hq_prev, c_prev, htag, ctag, also_bf16=False):
            """Encoder cell, fp8 DoubleRow matmuls, h stored as raw fp8."""
            e = cell_sb[nm]
            ps = PS()[:, :256]
            nc.tensor.matmul(ps, e["bB"], E_sb, start=True, stop=False)
            hv = hq_prev.rearrange("p (k s) -> p k s", s=BL)
            groups = [(e["whh"], hv, True)]
            if xq_ap is not None:
                groups.insert(0, (e["wih"], xq_ap, False))
            for w_sb, rhs, last in groups:
                for m in range(8):
                    nc.tensor.matmul(
                        ps[:, m * 32:(m + 1) * 32],
                        w_sb[:, :, m * 128:(m + 1) * 128],
                        rhs,
                        start=False, stop=last,
                        perf_mode=DR, skip_group_check=True,
                    )
            return cell_elem(ps, c_prev, htag, ctag, DT.float8e4,
                             1.0 / SCELL, also_bf16)  # -> finish()

        # ========== Phase D transition (callable mid-encoder) =============
        wd2_k = []
        pD_box = []

        def ensure_pD():
            if pD_box:
                return
            pA_ctx.close()  # free phase A SBUF before loading decoder weights
            pD = ctx.enter_context(tc.tile_pool(name="phaseD", bufs=1))
            pD_box.append(pD)
            for j in range(4):
                wk = pD.tile([128, 2, D], DT.bfloat16, tag=f"wd2_{j}",
                             name=f"wd2_{j}")
                nc.sync.dma_start(
                    wk[:], wd2T[j * 256:(j + 1) * 256, :]
                    .rearrange("(o p) m -> p o m", p=128))
                wd2_k.append(wk[:, 0])
                wd2_k.append(wk[:, 1])

        # ========== Encoder recurrence, pipelined w/ phase A fillers ======
        # Emission order per iteration: [filler, en3(t-1), en2(t), en1(t+1)].
        # Every cell's operands are already produced by chains that ran
        # during the preceding filler, so no blocked matmul ever sits in
        # front of ready work (PE wait queue is only 4 deep).
        needA(0)
        h1s, c1s = [None] * S, [None] * S
        h2s, c2s = [None] * S, [None] * S
        h3s, c3s = [None] * S, [None] * S

        def en3_p1(t):
            h3p = h3s[t - 1] if t else zh8
            c3p = c3s[t - 1] if t else zc
            x3 = h2s[t].rearrange("p (k s) -> p k s", s=BL)
            return cell_fp8("en3", x3, h3p, c3p, "h3", "c3")

        h1s[0], c1s[0], _ = cell_fp8("en1", z_ap(0), zh8, zc, "h1", "c1")()
        for t in range(S):
            fa, fb, fc_ = (2, 2, 1) if t < 5 else \
                ((2, 1, 1) if t < 10 else (1, 1, 1))
            if FILL_PLACE == "pre":
                fillA(fa + fb + fc_)
                fa = fb = fc_ = 0
            fillA(fa)
            fins = []
            if t + 1 < S:
                needA((t + 1) // SPC)
                fins.append((cell_fp8("en1", z_ap(t + 1), h1s[t], c1s[t],
                                      "h1", "c1"), "h1", t + 1))
            fillA(fb)
            h2p = h2s[t - 1] if t else zh8
            c2p = c2s[t - 1] if t else zc
            x2 = h1s[t].rearrange("p (k s) -> p k s", s=BL)
            fins.append((cell_fp8("en2", x2, h2p, c2p, "h2", "c2"), "h2", t))
            fillA(fc_)
            if t > 0:
                fins.append((en3_p1(t - 1), "h3", t - 1))
            for fin, which, tt in fins:
                h, c, _ = fin()
                if which == "h1":
                    h1s[tt], c1s[tt] = h, c
                elif which == "h2":
                    h2s[tt], c2s[tt] = h, c
                else:
                    h3s[tt], c3s[tt] = h, c
            if t == S - 5:
                needA(NCHA)  # drain phase A now so decoder weights can load
                ensure_pD()
        h3s[S - 1], c3s[S - 1], _ = en3_p1(S - 1)()

        ensure_pD()
        pD = pD_box[0]

        # ========== Phase D per-step generator ============================
        def phaseD_step(t, h3):
            """FC stack for decoder step t: h3 [128,64] bf16 -> yT cols."""
            h3v = h3.rearrange("p (k s) -> p k s", s=BL)
            psA = PS()[:, :256]
            for m in range(HID // 128):
                for k in range(2):
                    nc.tensor.matmul(
                        psA[:, m * 32:(m + 1) * 32],
                        wd1_sb[:, k, m * 128:(m + 1) * 128],
                        h3v[:, k, :],
                        start=(k == 0), stop=(k == 1),
                        skip_group_check=True,
                    )
            y1 = pD.tile([128, HID // 128, BL], DT.bfloat16,
                         tag="y1", name=f"y1_{t}")
            for m in range(HID // 128):
                epi_eng.tensor_scalar(
                    y1[:, m, :], psA[:, m * 32:(m + 1) * 32],
                    bd1_sb[:, m:m + 1], 0.0, ALU.add, ALU.max)
            yield
            o_all = outp.tile([128, 32 * BL], DT.float16, tag="o",
                              name=f"o{uid[0]}")
            for blk in range(2):
                psB = PSD()
                # bias preload: one K=128 indicator matmul fills all 16
                # m-slices, so the tanh epilogue is a single 512-col Act op
                nc.tensor.matmul(
                    psB[:], bd2B_sb[:, blk * 128:(blk + 1) * 128],
                    E16_sb, start=True, stop=False)
                for mi in range(16):
                    m = blk * 16 + mi
                    pm = psB[:, mi * 32:(mi + 1) * 32]
                    for k in range(HID // 128):
                        nc.tensor.matmul(
                            pm, wd2_k[k][:, m * 128:(m + 1) * 128],
                            y1[:, k, :],
                            start=False, stop=(k == HID // 128 - 1),
                            skip_group_check=True,
                        )
                    if mi % 4 == 3:
                        yield
                nc.scalar.activation(
                    o_all[:, blk * 512:(blk + 1) * 512], psB[:], AF.Tanh)
                yield
            nc.sync.dma_start(
                yT[:, t * BL:(t + 1) * BL].rearrange("(q p) s -> p q s", p=128),
                o_all.rearrange("p (q s) -> p q s", s=BL))

        genD_queue = []
        _SENT = object()

        def fillD(n=1):
            for _ in range(n):
                while genD_queue:
                    if next(genD_queue[0], _SENT) is _SENT:
                        genD_queue.pop(0)
                    else:
                        break

        # ========== Decoder recurrence, pipelined w/ phase D fillers ======
        # Same late-emission discipline as the encoder:
        # [filler, de3(t-1)+FCD(t-1), de2(t), de1(t+1)] per iteration.
        d1s, f1s = [None] * F, [None] * F
        d2s, f2s = [None] * F, [None] * F
        d3s, f3s = [None] * F, [None] * F

        d3bf = [None] * F

        def de3_p1(t):
            d3p = d3s[t - 1] if t else zh8
            f3p = f3s[t - 1] if t else zc
            x3 = d2s[t].rearrange("p (k s) -> p k s", s=BL)
            return cell_fp8("de3", x3, d3p, f3p, "d3", "e3", also_bf16=True)

        def de3_fin(fin, t):
            d3s[t], f3s[t], d3bf[t] = fin()
            genD_queue.append(phaseD_step(t, d3bf[t]))

        d1s[0], f1s[0], _ = cell_fp8("de1", None, h3s[S - 1], zc,
                                     "d1", "e1")()
        for t in range(F):
            fillD(5)
            fins = []
            if t + 1 < F:
                fins.append((cell_fp8("de1", None, d1s[t], f1s[t],
                                      "d1", "e1"), "d1", t + 1))
            d2p = d2s[t - 1] if t else zh8
            f2p = f2s[t - 1] if t else zc
            x2d = d1s[t].rearrange("p (k s) -> p k s", s=BL)
            fins.append((cell_fp8("de2", x2d, d2p, f2p, "d2", "e2"), "d2", t))
            if t > 0:
                fins.append((de3_p1(t - 1), "d3", t - 1))
            for fin, which, tt in fins:
                if which == "d1":
                    d1s[tt], f1s[tt], _ = fin()
                elif which == "d2":
                    d2s[tt], f2s[tt], _ = fin()
                else:
                    de3_fin(fin, tt)
            fillD(6)
        de3_fin(de3_p1(F - 1), F - 1)
        # drain remaining phase D work
        for gd in genD_queue:
            for _ in gd:
                pass

    nsplit = _split_sync_waits(nc, limit=1)
    _log(f"split {nsplit} over-limit sync waits")
    return nc

# ---------------------------------------------------------------------------
# Host-side input prep
# ---------------------------------------------------------------------------
GATE_PERM = np.concatenate([
    np.arange(0, 2 * H),          # i, f
    np.arange(3 * H, 4 * H),      # o
    np.arange(2 * H, 3 * H),      # g
])


def _kp(a):
    """[K, M] -> [128, K//256, 2, M] k-pair layout for DoubleRow."""
    k, m = a.shape
    return a.reshape(k // 256, 2, 128, m).transpose(2, 0, 1, 3)


def prep_inputs(inputs):
    f32 = np.float32
    g = {k: np.asarray(v) for k, v in inputs.items()}
    F = int(np.asarray(g["future_step"]))

    def bf(a):
        return np.ascontiguousarray(a).astype(BF16NP)

    def q8(a, scale):
        return (np.asarray(a, f32) * scale).astype(FP8NP)

    shared = {}
    shared["w1q"] = np.ascontiguousarray(_kp(q8(g["fc_en1_w"].T, SW1)))
    shared["w2q"] = np.ascontiguousarray(_kp(q8(g["fc_en2_w"].T, SW2)))
    # double the g-gate rows (perm'd positions 3H:4H) so tanh(pre_g) can be
    # recovered from sigmoid(2*pre_g) computed in the single per-cell sigmoid
    GDOUBLE = np.ones((G, 1), f32)
    GDOUBLE[3 * H:] = 2.0
    bBs = []
    for nm in EN_CELLS:
        wih = g[nm + "_wih"][GATE_PERM] * GDOUBLE
        whh = g[nm + "_whh"][GATE_PERM] * GDOUBLE
        bsum = ((g[nm + "_bih"] + g[nm + "_bhh"])[GATE_PERM].astype(f32)
                * GDOUBLE[:, 0])
        s_wih = SW1 if nm == "en1" else SCELL  # en1 x-side is zq (SZ=64)
        shared[nm + "_w"] = np.ascontiguousarray(np.stack(
            [_kp(q8(wih.T, s_wih))[:, 0], _kp(q8(whh.T, SCELL))[:, 0]],
            axis=1))
        bB = np.zeros((128, 128), f32)
        bB[:G // 128, :] = SCELL * bsum.reshape(G // 128, 128)
        bBs.append(bB)
    dew_parts = []
    for nm in DE_CELLS:
        wih = g[nm + "_wih"][GATE_PERM] * GDOUBLE
        whh = g[nm + "_whh"][GATE_PERM] * GDOUBLE
        bsum = ((g[nm + "_bih"] + g[nm + "_bhh"])[GATE_PERM].astype(f32)
                * GDOUBLE[:, 0])
        if nm != "de1":
            dew_parts.append(_kp(q8(wih.T, SCELL))[:, 0])
        dew_parts.append(_kp(q8(whh.T, SCELL))[:, 0])
        bB = np.zeros((128, 128), f32)
        bB[:G // 128, :] = SCELL * bsum.reshape(G // 128, 128)
        bBs.append(bB)
    # pack order: de1_whh, de2_wih, de2_whh, de3_wih, de3_whh
    shared["dew"] = np.ascontiguousarray(np.stack(dew_parts, axis=1))
    shared["wd1m"] = np.ascontiguousarray(_kp(bf(g["fc_de1_w"].T))[:, 0])

    E = np.zeros((128, 256), f32)
    for j in range(8):
        E[j, j * 32:(j + 1) * 32] = 1.0
    E16 = np.zeros((128, 512), f32)
    for j in range(16):
        E16[j, j * 32:(j + 1) * 32] = 1.0
    bd2 = g["fc_de2_b"].astype(f32)       # [4096] = 2 blocks x 16 m x 128
    bd2B = np.zeros((128, 256), f32)
    for blk in range(2):
        bd2B[:16, blk * 128:(blk + 1) * 128] = \
            bd2[blk * 2048:(blk + 1) * 2048].reshape(16, 128)
    shared["cbf"] = np.ascontiguousarray(np.concatenate(
        [E, E16] + bBs + [bd2B], axis=1)).astype(BF16NP)
    shared["cf32"] = np.ascontiguousarray(np.concatenate([
        (SZ1 * g["fc_en1_b"]).astype(f32).reshape(HID // 128, 128).T,
        (SZ * g["fc_en2_b"]).astype(f32).reshape(H // 128, 128).T,
        g["fc_de1_b"].astype(f32).reshape(HID // 128, 128).T,
    ], axis=1))
    shared["wd2T"] = bf(g["fc_de2_w"].T)

    x = g["x"].astype(f32).reshape(S, B, D)
    in_maps = []
    for c in range(NCORES):
        xc = x[:, c * BL:(c + 1) * BL, :].reshape(SB, D)   # row = t*BL + b
        m = dict(shared)
        # [128, NCHA, KP1, 2, CHA]: x[p, c, kp, o, n] = xcT[kp*256+o*128+p,
        # c*CHA+n]
        xq8 = q8(xc.T, 1.0).reshape(KP1, 2, 128, NCHA, CHA)
        m["xq"] = np.ascontiguousarray(xq8.transpose(2, 3, 0, 1, 4))
        in_maps.append(m)
    return in_maps, F


# ---------------------------------------------------------------------------
# Execution via PJRT (axon), modeled on bass2jax.run_bass_via_pjrt
# ---------------------------------------------------------------------------
def run_spmd(nc, in_maps, n_timing=0):
    import jax
    from jax.experimental.shard_map import shard_map
    from jax.sharding import Mesh, NamedSharding, PartitionSpec

    from concourse import bass2jax

    bass2jax.install_neuronx_cc_hook()
    n_cores = len(in_maps)
    partition_name = nc.partition_id_tensor.name if nc.partition_id_tensor else None
    in_names, out_names, out_avals, zero_outs = [], [], [], []
    for alloc in nc.m.functions[0].allocations:
        if not isinstance(alloc, mybir.MemoryLocationSet):
            continue
        name = alloc.memorylocations[0].name
        if alloc.kind == "ExternalInput":
            if name != partition_name:
                in_names.append(name)
        elif alloc.kind == "ExternalOutput":
            out_names.append(name)
            shape = tuple(alloc.tensor_shape)
            dtype = mybir.dt.np(alloc.dtype)
            out_avals.append(jax.core.ShapedArray(shape, dtype))
            zero_outs.append(np.zeros(shape, dtype))
    n_params = len(in_names)
    all_in = in_names + out_names
    if partition_name is not None:
        all_in = all_in + [partition_name]
    all_in = tuple(all_in)

    def _bind(args):
        operands = list(args)
        if partition_name is not None:
            operands.append(bass2jax.partition_id_tensor())
        return bass2jax._bass_exec_p.bind(
            *operands,
            out_avals=tuple(out_avals),
            in_names=all_in,
            out_names=tuple(out_names),
            lowering_input_output_aliases=(),
            sim_require_finite=False,
            sim_require_nnan=False,
            nc=nc,
        )

    def _body(*args):
        return tuple(_bind(args))

    devices = jax.devices()[:n_cores]
    mesh = Mesh(np.asarray(devices), ("core",))
    pspec = PartitionSpec("core")
    in_specs = (pspec,) * (n_params + len(out_names))
    out_specs = (pspec,) * len(out_names)

    f1 = jax.jit(shard_map(_body, mesh=mesh, in_specs=in_specs,
                           out_specs=out_specs, check_rep=False))
    concat = [
        np.concatenate([np.asarray(in_maps[c][nm]) for c in range(n_cores)], axis=0)
        for nm in in_names
    ]
    concat += [np.concatenate([z] * n_cores, axis=0) for z in zero_outs]

    sharding = NamedSharding(mesh, pspec)
    t0 = time.perf_counter()
    dev_in = [jax.device_put(a, sharding) for a in concat]
    jax.block_until_ready(dev_in)
    _log(f"upload {sum(a.nbytes for a in concat)/1e6:.1f} MB in "
         f"{time.perf_counter()-t0:.2f}s")

    t0 = time.perf_counter()
    outs = jax.block_until_ready(f1(*dev_in))
    _log(f"first run (incl compile) {time.perf_counter()-t0:.1f}s")

    results = []
    np_outs = [np.asarray(o) for o in outs]
    for c in range(n_cores):
        r = {}
        for i, nm in enumerate(out_names):
            sh0 = out_avals[i].shape[0]
            r[nm] = np_outs[i][c * sh0:(c + 1) * sh0]
        results.append(r)

    wall = None
    if n_timing:
        ts = []
        for _ in range(n_timing):
            t0 = time.perf_counter()
            jax.block_until_ready(f1(*dev_in))
            ts.append(time.perf_counter() - t0)
        wall = min(ts)
        _log("wall per call ms: " + " ".join(f"{t*1e3:.2f}" for t in ts))
    return results, wall, (f1, dev_in)


def measure_hw_time(F, in_maps, nrep=9, reps=14):
    """HW exec estimate: (wall(nrep-program) - wall(1-program)) / (nrep-1),
    with the two programs timed in an interleaved loop to cancel drift.
    Host-side RPC jitter is ~1ms so this is accurate to roughly +-50us."""
    import jax

    nc1 = build_program(F, nrep=1)
    _, _, (f1, dev1) = run_spmd(nc1, in_maps)
    ncN = build_program(F, nrep=nrep)
    _, _, (fN, devN) = run_spmd(ncN, in_maps)
    t1s, tNs = [], []
    for _ in range(reps):
        t0 = time.perf_counter()
        jax.block_until_ready(f1(*dev1))
        t1s.append(time.perf_counter() - t0)
        t0 = time.perf_counter()
        jax.block_until_ready(fN(*devN))
        tNs.append(time.perf_counter() - t0)
    w1, wN = min(t1s), min(tNs)
    per_iter = (wN - w1) / (nrep - 1)
    _log("t1 ms: " + " ".join(f"{t*1e3:.2f}" for t in t1s))
    _log(f"t{nrep} ms: " + " ".join(f"{t*1e3:.2f}" for t in tNs))
    _log(f"measure: w1={w1*1e3:.3f}ms w{nrep}={wN*1e3:.3f}ms -> "
         f"{per_iter*1e6:.1f}us/iter")
    return per_iter * 1e9


def kernel(**inputs) -> np.ndarray:
    t0 = time.perf_counter()
    in_maps, F = prep_inputs(inputs)
    _log(f"host prep {time.perf_counter()-t0:.2f}s")
    t0 = time.perf_counter()
    nc = build_program(F)
    _log(f"build+tile {time.perf_counter()-t0:.1f}s")
    results, _, _ = run_spmd(nc, in_maps)
    out = np.empty((F, B, 64, 64), np.float32)
    for c in range(NCORES):
        yT = results[c]["yT"]                      # [4096, F*32] fp16
        y = yT.astype(np.float32).T.reshape(F, BL, 64, 64)
        out[:, c * BL:(c + 1) * BL] = y
    return out
